# revision 12
# baseline (speedup 1.0000x reference)
"""Segment mean-pool (BERT lattice embedding) Trainium2 Bass kernel.

Full-input contract: kernel(hidden[64,512,768] f32, word_ids[64,512] i32,
num_tokens=400) -> [64,400,768] f32.

Strategy: data-parallel over batch across 8 NeuronCores (8 samples each).
word_ids are NON-DECREASING per sample (HF tokenizer word_ids()), so the
128 pieces of chunk j = [128j, 128j+128) map into a narrow word window
[base_j, base_j + U) with base_j = word_ids[b, 128j] and U = 128 covering
the measured max window width. Per (sample, chunk) the ragged segment MEAN
is ONE 128x128 scaled-one-hot matmul per PSUM bank:

    A_j[p, u]   = (word_ids[b,128j+p] - base_j == u) * recip[wid[b,128j+p]]
                  (one fused is_equal+mult tensor_scalar per chunk)
    psum_j[u,:] = A_j.T @ hidden[b, 128j:128j+128, :]     u = t - base_j

Row p of A_j has a single nonzero at u = its own word offset, scaled by
that word's 1/max(count,1) - so psum already holds the final mean and the
post-matmul step is a pure PSUM->SBUF bf16 cast, spread across ACT / DVE /
GpSimd. Every hidden element enters the PE exactly once (3072 streamed
cols/sample) and everything runs in bf16 (~3e-3 rel err vs the 2e-2 gate):
input DMA halves to 6.3 MB/core and the PE runs at full bf16 rate. The
device emits per-chunk windows [B_LOC, 128, J, H] bf16; the host adds
windows into the final [400] rows (consecutive windows overlap in at most
the boundary word, and both partials carry the same per-word 1/count
scale, so plain addition is exact).

Chunk windows wider than U (impossible for the staged distribution, checked
at run time) trigger a rebuild with U=256 (two M-tiles per chunk).

DMA rings: bulk input prefetch on the sync HWDGE ring (6 KB descriptors,
whole shard fits SBUF; sample 0 split per chunk to start compute early),
outputs + index tensors on the gpsimd ring (the tiny index loads at t=0
also warm the ring), keeping the ACT engine free for PSUM casts.
"""

import numpy as np
import ml_dtypes

B, S, H, T = 64, 512, 768, 400
N_CORES = 8
B_LOC = B // N_CORES  # samples per core
P = 128
J = S // P  # piece chunks per sample
N0 = 384  # h split: two psum banks
U_DEFAULT = 128

BF16 = ml_dtypes.bfloat16

_CACHED = {}


def build_program(u_width=U_DEFAULT):
    """Build + compile the single-core Bass program (same NEFF on all cores)."""
    import concourse.bass as bass  # noqa: F401
    import concourse.mybir as mybir
    import concourse.tile as tile
    from concourse import bacc

    n_mt = u_width // P  # M-tiles per chunk window
    assert u_width % P == 0

    nc = bacc.Bacc(
        "TRN2",
        target_bir_lowering=False,
        debug=False,
        enable_asserts=False,
        num_devices=N_CORES,
    )
    f32 = mybir.dt.float32
    bf16 = mybir.dt.bfloat16

    # hid_pjh[b, p, j, :] = hidden[b, 128j + p, :] in bf16 (host-packed so
    # every partition reads one contiguous J*H run per sample).
    hid_t = nc.dram_tensor("hid_pjh", [B_LOC, P, J, H], bf16, kind="ExternalInput").ap()
    # widl[p, b, j] = word_ids[b, 128j+p] - base[b, j]  (f32, values 0..U-1)
    widl_t = nc.dram_tensor("wid_local", [P, B_LOC, J], f32, kind="ExternalInput").ap()
    # recp[p, b, j] = 1/max(count[b, word_ids[b, 128j+p]], 1)  (per-piece)
    recp_t = nc.dram_tensor("recip_pc", [P, B_LOC, J], f32, kind="ExternalInput").ap()
    # out[b, mt, u, j, :] = window row u of chunk j (word base[b,j]+128*mt+u)
    out_t = nc.dram_tensor(
        "out_loc", [B_LOC, n_mt, P, J, H], bf16, kind="ExternalOutput"
    ).ap()

    with tile.TileContext(nc) as tc:
        with tc.tile_pool(name="const", bufs=1) as const_pool, \
             tc.tile_pool(name="hidp", bufs=B_LOC) as hid_pool, \
             tc.tile_pool(name="aTp", bufs=3) as aT_pool, \
             tc.tile_pool(name="outp", bufs=3) as out_pool, \
             tc.tile_pool(name="psum", bufs=4, space="PSUM") as psum_pool:

            # Bulk input prefetch first: the sync ring streams the whole
            # shard back-to-back from t=0. Sample 0 split per j-chunk so the
            # first matmul can start as soon as chunk 0 lands.
            hids = []
            for b in range(B_LOC):
                hid = hid_pool.tile([P, J, H], bf16, name=f"hid{b}", tag="hid")
                if b == 0:
                    for j in range(J):
                        nc.sync.dma_start(out=hid[:, j, :], in_=hid_t[b, :, j, :])
                else:
                    nc.sync.dma_start(out=hid, in_=hid_t[b])
                hids.append(hid)

            iota_t = const_pool.tile([P, u_width], f32, name="iota_t")
            nc.gpsimd.iota(
                iota_t,
                pattern=[[1, u_width]],
                base=0,
                channel_multiplier=0,
                allow_small_or_imprecise_dtypes=True,
            )
            # Index tensors: tiny, land early, and one on each of the two
            # rings the outputs will use (warms both HWDGE rings at t=0).
            widl_sb = const_pool.tile([P, B_LOC, J], f32, name="widl_sb")
            nc.scalar.dma_start(out=widl_sb, in_=widl_t)
            recp_sb = const_pool.tile([P, B_LOC, J], f32, name="recp_sb")
            nc.gpsimd.dma_start(out=recp_sb, in_=recp_t)

            for b in range(B_LOC):
                hid = hids[b]
                aT = aT_pool.tile([P, J, u_width], bf16, name="aT", tag="aT")
                for j in range(J):
                    # Scaled one-hot: (iota == widl) * recip_piece, fused,
                    # on GpSimd (SBUF-only reads; keeps ACT/DVE for casts).
                    nc.gpsimd.tensor_scalar(
                        aT[:, j, :],
                        iota_t,
                        widl_sb[:, b, j : j + 1],
                        recp_sb[:, b, j : j + 1],
                        op0=mybir.AluOpType.is_equal,
                        op1=mybir.AluOpType.mult,
                    )
                oms = [
                    out_pool.tile([P, J, H], bf16, name=f"om{mt}", tag=f"om{mt}")
                    for mt in range(n_mt)
                ]
                for j in range(J):
                    for mt in range(n_mt):
                        om = oms[mt]
                        ps0 = psum_pool.tile([P, N0], f32, name="ps0", tag="ps0")
                        ps1 = psum_pool.tile([P, N0], f32, name="ps1", tag="ps1")
                        lhsT = aT[:, j, mt * P : (mt + 1) * P]
                        nc.tensor.matmul(ps0, lhsT, hid[:, j, 0:N0], start=True, stop=True)
                        nc.tensor.matmul(ps1, lhsT, hid[:, j, N0:H], start=True, stop=True)
                        # PSUM -> SBUF bf16 cast, spread across three engines:
                        # ACT always takes bank 0; DVE / GpSimd alternate bank 1.
                        nc.scalar.mul(om[:, j, 0:N0], ps0, 1.0)
                        nc.vector.tensor_scalar_mul(om[:, j, N0:H], ps1, 1.0)
                    # Stream each half-sample as soon as it is cast - the
                    # two output HWDGE rings (scalar / gpsimd) alternate,
                    # both separate FIFOs from the input prefetch.
                    if j % 2 == 1:
                        eng = nc.scalar if j == 1 else nc.gpsimd
                        for mt in range(n_mt):
                            eng.dma_start(
                                out=out_t[b, mt, :, j - 1 : j + 1, :],
                                in_=oms[mt][:, j - 1 : j + 1, :],
                            )

    nc.compile()
    return nc


def _pack_inputs(hidden, word_ids, u_width):
    """Full-batch host prep: bf16 cast + per-core input maps."""
    hidden = np.ascontiguousarray(np.asarray(hidden), dtype=np.float32).reshape(B, S, H)
    wid = np.ascontiguousarray(np.asarray(word_ids), dtype=np.int32).reshape(B, S)

    hid16 = hidden.astype(BF16)

    counts = np.zeros((B, T), np.int64)
    np.add.at(counts, (np.repeat(np.arange(B), S), wid.reshape(-1)), 1)
    recip = (1.0 / np.maximum(counts, 1)).astype(np.float32)  # [B, T]
    # per-piece recip: rp[b, s] = recip[b, wid[b, s]]
    rp = np.take_along_axis(recip, wid, axis=1)  # [B, S]

    base = wid[:, ::P]  # [B, J] first word id of each chunk

    in_maps = []
    for i in range(N_CORES):
        sl = slice(i * B_LOC, (i + 1) * B_LOC)
        hs = np.ascontiguousarray(
            hid16[sl].reshape(B_LOC, J, P, H).transpose(0, 2, 1, 3)
        )
        wl = np.ascontiguousarray(
            (wid[sl].reshape(B_LOC, J, P) - base[sl][:, :, None])
            .transpose(2, 0, 1)
            .astype(np.float32)
        )
        rc = np.ascontiguousarray(
            rp[sl].reshape(B_LOC, J, P).transpose(2, 0, 1).astype(np.float32)
        )
        in_maps.append({"hid_pjh": hs, "wid_local": wl, "recip_pc": rc})
    return in_maps


def _combine(core_outs, word_ids, u_width):
    """Scatter-add per-chunk windows into the full [B, T, H] f32 output."""
    wid = np.asarray(word_ids, np.int32).reshape(B, S)
    base = wid[:, ::P]  # [B, J]
    out = np.zeros((B, T, H), np.float32)
    for i, arr in enumerate(core_outs):
        # arr: [B_LOC, n_mt, P, J, H] bf16 -> [B_LOC, U, J, H] f32
        a = np.asarray(arr).astype(np.float32).reshape(B_LOC, u_width, J, H)
        for b in range(B_LOC):
            gb = i * B_LOC + b
            for j in range(J):
                t0 = int(base[gb, j])
                w = min(u_width, T - t0)
                out[gb, t0 : t0 + w] += a[b, :w, j]
    return out


def _u_required(word_ids):
    wid = np.asarray(word_ids, np.int32).reshape(B, S)
    wmax = 0
    for j in range(J):
        wmax = max(wmax, int((wid[:, (j + 1) * P - 1] - wid[:, j * P]).max()) + 1)
    return -(-wmax // P) * P  # round up to multiple of 128


def run(hidden, word_ids, trace=False, **trace_kwargs):
    from concourse import bass_utils

    u_width = max(U_DEFAULT, _u_required(word_ids))
    if u_width not in _CACHED:
        _CACHED[u_width] = build_program(u_width)
    nc = _CACHED[u_width]
    in_maps = _pack_inputs(hidden, word_ids, u_width)
    res = bass_utils.run_bass_kernel_spmd(
        nc, in_maps, core_ids=list(range(N_CORES)), trace=trace, **trace_kwargs
    )
    out = _combine(
        [res.results[i]["out_loc"] for i in range(N_CORES)], word_ids, u_width
    )
    return out, res


def kernel(hidden, word_ids, num_tokens=None, **_unused):
    out, _ = run(hidden, word_ids, trace=False)
    return out


# revision 17
# speedup vs baseline: 1.8671x; 1.8671x over previous
"""Segment mean-pool (BERT lattice embedding) Trainium2 Bass kernel.

Full-input contract: kernel(hidden[64,512,768] f32, word_ids[64,512] i32,
num_tokens=400) -> [64,400,768] f32.

Strategy: data-parallel over batch across 8 NeuronCores (8 samples each).
word_ids are NON-DECREASING per sample (HF tokenizer word_ids()), so the
128 pieces of chunk j = [128j, 128j+128) map into a narrow word window
[base_j, base_j + U) with base_j = word_ids[b, 128j] and U = 128 covering
the measured max window width. Per (sample, chunk) the ragged segment MEAN
is ONE 128x128 scaled-one-hot matmul per PSUM bank:

    A_j[p, u]   = (word_ids[b,128j+p] - base_j == u) * recip[wid[b,128j+p]]
                  (one fused is_equal+mult tensor_scalar per chunk)
    psum_j[u,:] = A_j.T @ hidden[b, 128j:128j+128, :]     u = t - base_j

Row p of A_j has a single nonzero at u = its own word offset, scaled by
that word's 1/max(count,1) - so psum already holds the final mean and the
post-matmul step is a pure PSUM->SBUF bf16 cast, spread across ACT / DVE /
GpSimd. Every hidden element enters the PE exactly once (3072 streamed
cols/sample) and everything runs in bf16 (~3e-3 rel err vs the 2e-2 gate):
input DMA halves to 6.3 MB/core and the PE runs at full bf16 rate. The
device emits per-chunk windows [B_LOC, 128, J, H] bf16; the host adds
windows into the final [400] rows (consecutive windows overlap in at most
the boundary word, and both partials carry the same per-word 1/count
scale, so plain addition is exact).

Chunk windows wider than U (impossible for the staged distribution, checked
at run time) trigger a rebuild with U=256 (two M-tiles per chunk).

DMA rings: bulk input prefetch on the sync HWDGE ring (6 KB descriptors,
whole shard fits SBUF; sample 0 split per chunk to start compute early),
outputs + index tensors on the gpsimd ring (the tiny index loads at t=0
also warm the ring), keeping the ACT engine free for PSUM casts.
"""

import numpy as np
import ml_dtypes

B, S, H, T = 64, 512, 768, 400
N_CORES = 8
B_LOC = B // N_CORES  # samples per core
P = 128
J = S // P  # piece chunks per sample
N0 = 448  # h split: two psum banks; ACT casts [0:448], DVE casts [448:768]
           # (DVE also builds the one-hots, so it gets the smaller slice)
U_DEFAULT = 128

BF16 = ml_dtypes.bfloat16

_CACHED = {}


def build_program(u_width=U_DEFAULT):
    """Build + compile the single-core Bass program (same NEFF on all cores)."""
    import concourse.bass as bass  # noqa: F401
    import concourse.mybir as mybir
    import concourse.tile as tile
    from concourse import bacc

    n_mt = u_width // P  # M-tiles per chunk window
    assert u_width % P == 0

    nc = bacc.Bacc(
        "TRN2",
        target_bir_lowering=False,
        debug=False,
        enable_asserts=False,
        num_devices=N_CORES,
    )
    f32 = mybir.dt.float32
    bf16 = mybir.dt.bfloat16

    # hid_pjh[b, p, j, :] = hidden[b, 128j + p, :] in bf16 (host-packed so
    # every partition reads one contiguous J*H run per sample).
    hid_t = nc.dram_tensor("hid_pjh", [B_LOC, P, J, H], bf16, kind="ExternalInput").ap()
    # widl[p, b, j] = word_ids[b, 128j+p] - base[b, j]  (f32, values 0..U-1)
    widl_t = nc.dram_tensor("wid_local", [P, B_LOC, J], f32, kind="ExternalInput").ap()
    # recp[p, b, j] = 1/max(count[b, word_ids[b, 128j+p]], 1)  (per-piece)
    recp_t = nc.dram_tensor("recip_pc", [P, B_LOC, J], f32, kind="ExternalInput").ap()
    # out[b, mt, u, j, :] = window row u of chunk j (word base[b,j]+128*mt+u)
    out_t = nc.dram_tensor(
        "out_loc", [B_LOC, n_mt, P, J, H], bf16, kind="ExternalOutput"
    ).ap()

    with tile.TileContext(nc) as tc:
        with tc.tile_pool(name="const", bufs=1) as const_pool, \
             tc.tile_pool(name="hidp", bufs=B_LOC) as hid_pool, \
             tc.tile_pool(name="aTp", bufs=3) as aT_pool, \
             tc.tile_pool(name="outp", bufs=3) as out_pool, \
             tc.tile_pool(name="psum", bufs=4, space="PSUM") as psum_pool:

            # Bulk input prefetch first: the sync ring streams the whole
            # shard back-to-back from t=0. Sample 0 split per j-chunk so the
            # first matmul can start as soon as chunk 0 lands.
            hids = []
            for b in range(B_LOC):
                hid = hid_pool.tile([P, J, H], bf16, name=f"hid{b}", tag="hid")
                if b == 0:
                    for j in range(J):
                        nc.sync.dma_start(out=hid[:, j, :], in_=hid_t[b, :, j, :])
                else:
                    nc.sync.dma_start(out=hid, in_=hid_t[b])
                hids.append(hid)

            iota_t = const_pool.tile([P, u_width], f32, name="iota_t")
            nc.gpsimd.iota(
                iota_t,
                pattern=[[1, u_width]],
                base=0,
                channel_multiplier=0,
                allow_small_or_imprecise_dtypes=True,
            )
            # Index tensors: tiny, land early on the gpsimd ring - the same
            # ring the outputs use, so it is warm before the first drain.
            widl_sb = const_pool.tile([P, B_LOC, J], f32, name="widl_sb")
            nc.gpsimd.dma_start(out=widl_sb, in_=widl_t)
            recp_sb = const_pool.tile([P, B_LOC, J], f32, name="recp_sb")
            nc.gpsimd.dma_start(out=recp_sb, in_=recp_t)

            for b in range(B_LOC):
                hid = hids[b]
                aT = aT_pool.tile([P, J, u_width], bf16, name="aT", tag="aT")
                for j in range(J):
                    # Scaled one-hot: (iota == widl) * recip_piece, fused.
                    nc.vector.tensor_scalar(
                        aT[:, j, :],
                        iota_t,
                        widl_sb[:, b, j : j + 1],
                        recp_sb[:, b, j : j + 1],
                        op0=mybir.AluOpType.is_equal,
                        op1=mybir.AluOpType.mult,
                    )
                oms = [
                    out_pool.tile([P, J, H], bf16, name=f"om{mt}", tag=f"om{mt}")
                    for mt in range(n_mt)
                ]
                for j in range(J):
                    for mt in range(n_mt):
                        om = oms[mt]
                        ps0 = psum_pool.tile([P, N0], f32, name="ps0", tag="ps0")
                        ps1 = psum_pool.tile([P, H - N0], f32, name="ps1", tag="ps1")
                        lhsT = aT[:, j, mt * P : (mt + 1) * P]
                        nc.tensor.matmul(ps0, lhsT, hid[:, j, 0:N0], start=True, stop=True)
                        nc.tensor.matmul(ps1, lhsT, hid[:, j, N0:H], start=True, stop=True)
                        # PSUM -> SBUF bf16 cast, spread across three engines:
                        # ACT always takes bank 0; DVE / GpSimd alternate bank 1.
                        nc.scalar.mul(om[:, j, 0:N0], ps0, 1.0)
                        nc.vector.tensor_scalar_mul(om[:, j, N0:H], ps1, 1.0)
                    # Stream each half-sample as soon as it is cast - gpsimd
                    # HWDGE ring, separate FIFO from the input prefetch, and
                    # issued off-engine so ACT/DVE stay on casts.
                    if j % 2 == 1:
                        for mt in range(n_mt):
                            nc.gpsimd.dma_start(
                                out=out_t[b, mt, :, j - 1 : j + 1, :],
                                in_=oms[mt][:, j - 1 : j + 1, :],
                            )

    nc.compile()
    return nc


def _pack_inputs(hidden, word_ids, u_width):
    """Full-batch host prep: bf16 cast + per-core input maps."""
    hidden = np.ascontiguousarray(np.asarray(hidden), dtype=np.float32).reshape(B, S, H)
    wid = np.ascontiguousarray(np.asarray(word_ids), dtype=np.int32).reshape(B, S)

    hid16 = hidden.astype(BF16)

    counts = np.zeros((B, T), np.int64)
    np.add.at(counts, (np.repeat(np.arange(B), S), wid.reshape(-1)), 1)
    recip = (1.0 / np.maximum(counts, 1)).astype(np.float32)  # [B, T]
    # per-piece recip: rp[b, s] = recip[b, wid[b, s]]
    rp = np.take_along_axis(recip, wid, axis=1)  # [B, S]

    base = wid[:, ::P]  # [B, J] first word id of each chunk

    in_maps = []
    for i in range(N_CORES):
        sl = slice(i * B_LOC, (i + 1) * B_LOC)
        hs = np.ascontiguousarray(
            hid16[sl].reshape(B_LOC, J, P, H).transpose(0, 2, 1, 3)
        )
        wl = np.ascontiguousarray(
            (wid[sl].reshape(B_LOC, J, P) - base[sl][:, :, None])
            .transpose(2, 0, 1)
            .astype(np.float32)
        )
        rc = np.ascontiguousarray(
            rp[sl].reshape(B_LOC, J, P).transpose(2, 0, 1).astype(np.float32)
        )
        in_maps.append({"hid_pjh": hs, "wid_local": wl, "recip_pc": rc})
    return in_maps


def _combine(core_outs, word_ids, u_width):
    """Scatter-add per-chunk windows into the full [B, T, H] f32 output."""
    wid = np.asarray(word_ids, np.int32).reshape(B, S)
    base = wid[:, ::P]  # [B, J]
    out = np.zeros((B, T, H), np.float32)
    for i, arr in enumerate(core_outs):
        # arr: [B_LOC, n_mt, P, J, H] bf16 -> [B_LOC, U, J, H] f32
        a = np.asarray(arr).astype(np.float32).reshape(B_LOC, u_width, J, H)
        for b in range(B_LOC):
            gb = i * B_LOC + b
            for j in range(J):
                t0 = int(base[gb, j])
                w = min(u_width, T - t0)
                out[gb, t0 : t0 + w] += a[b, :w, j]
    return out


def _u_required(word_ids):
    wid = np.asarray(word_ids, np.int32).reshape(B, S)
    wmax = 0
    for j in range(J):
        wmax = max(wmax, int((wid[:, (j + 1) * P - 1] - wid[:, j * P]).max()) + 1)
    return -(-wmax // P) * P  # round up to multiple of 128


def run(hidden, word_ids, trace=False, **trace_kwargs):
    from concourse import bass_utils

    u_width = max(U_DEFAULT, _u_required(word_ids))
    if u_width not in _CACHED:
        _CACHED[u_width] = build_program(u_width)
    nc = _CACHED[u_width]
    in_maps = _pack_inputs(hidden, word_ids, u_width)
    res = bass_utils.run_bass_kernel_spmd(
        nc, in_maps, core_ids=list(range(N_CORES)), trace=trace, **trace_kwargs
    )
    out = _combine(
        [res.results[i]["out_loc"] for i in range(N_CORES)], word_ids, u_width
    )
    return out, res


def kernel(hidden, word_ids, num_tokens=None, **_unused):
    out, _ = run(hidden, word_ids, trace=False)
    return out
